# revision 11
# baseline (speedup 1.0000x reference)
"""TRN2 Bass kernel for nn_CombCrossAttention (GQA cross-attention block).

Computation (T=2048, K=2048, E=4096, H=32 q-heads, KVH=8 kv-heads, D=128):
    q  = hidden @ Wq.T;  per-head RMSNorm(q) * q_norm_w
    kn = RMSNorm(k) * k_norm_w  (GQA: each kv head serves 4 q heads)
    attn = softmax(qn @ kn.T / sqrt(D)) @ v
    out  = attn @ Wo.T

Sharding: tensor-parallel over heads on 8 NeuronCores. Core c owns q-heads
4c..4c+3 (Wq rows 512c..512c+512) and kv-head c, plus Wo columns
512c..512c+512; each core emits a [T, E] partial of the o-projection and
the host sums the 8 partials (the "all-reduce").

Fully-fused single pipeline over t-chunks of 512 (tcn = 0..3):
    qproj(tcn) -> attention(tcn) [+ oproj(tcn-1) interleaved] -> norm(tcn)
with the last oproj trailing. Everything is computed transposed
([feature, t]) so no on-chip transposes are needed.

Per-core engine budget (~2.15 GHz PE, 1 cycle/row, ~238 ns per 512-row
matmul): PE runs 1568 matmuls ~ 373 us and is the wall. The softmax
denominator is kept off the PE (vs 256 ones-matmuls in the naive scheme):
exp tiles are pair-summed in bf16 (GpSimd) and accumulated in f32 (DVE),
with one [128,512] ones-matmul per (head, tcn) for the cross-partition
sum, and a fast-approx DVE reciprocal (the iterative DVE reciprocal is
5x slower; ACT reciprocal would force an activation-table switch).
The q-RMSNorm rsqrt runs as exp(-0.5*ln(x)) on ACT -- ln/exp share one
activation table set with the attention exp, so the table loads once
(the v1 kernel paid 29 table loads / 37 us by also using ACT Square).

PSUM (8 banks) is time-shared via pool tags:
  acc  4x[128,512]  attention AV accumulators <-> qproj psum tiles
  pp2  2x[128,512]  interleaved-oproj tiles <-> RMS sums <-> denominators
  scr  2x[128,512]  score tiles; exp WAR-serializes slot reuse, which
       leaves PE idle slots that the interleaved oproj matmuls fill.

bf16 is used where the error budget (2e-2) allows: hidden/Wq (q-proj),
v / exp tiles (AV matmul), Wo / attn-out (o-proj), and the output
partials; scores and k'' stay f32r. All matmuls run at 1 cycle/row.
"""
import sys

sys.path.insert(0, "/opt/trn_rl_repo")

import numpy as np
import ml_dtypes

import jax
try:
    jax.config.update("jax_compilation_cache_dir", "/tmp/jax_neff_cache")
    jax.config.update("jax_persistent_cache_min_compile_time_secs", 1.0)
except Exception:
    pass

import concourse.bass as bass  # noqa: F401
import concourse.mybir as mybir
import concourse.tile as tile
from concourse import bacc, bass_utils

EPS = 1e-5
T, K, E, H, KVH, D = 2048, 2048, 4096, 32, 8, 128
N_CORES = 8
HL = H // N_CORES      # 4 q-heads per core
EL = HL * D            # 512 local embed columns
NT = 4                 # t-chunks of 512
f32 = mybir.dt.float32
f32r = mybir.dt.float32r
bf16 = mybir.dt.bfloat16
BF = ml_dtypes.bfloat16

Ln = mybir.ActivationFunctionType.Ln
Exp = mybir.ActivationFunctionType.Exp


def _kernel_body(tc):
    nc = tc.nc
    # hid: [tcn][group][partition][kt-in-group][t] bf16
    hid = nc.dram_tensor("hid", [NT, 4, 128, 8, 512], bf16, kind="ExternalInput").ap()
    # wq: [m][partition(e-chunk)][kt][col] bf16 lhsT tiles
    wq = nc.dram_tensor("wq", [4, 128, 32, 128], bf16, kind="ExternalInput").ap()
    kpp = nc.dram_tensor("kpp", [128, 2048], f32r, kind="ExternalInput").ap()
    vt = nc.dram_tensor("vt", [128, 16, 128], bf16, kind="ExternalInput").ap()
    # wo: [hq][partition(d)][mo][col] bf16 lhsT tiles
    wo = nc.dram_tensor("wo", [4, 128, 32, 128], bf16, kind="ExternalInput").ap()
    onesd = nc.dram_tensor("ones", [128, 128], f32r, kind="ExternalInput").ap()
    # outp: [tcn][mg][partition][mj][t] bf16 (e_global = (mg*4+mj)*128 + p)
    outp = nc.dram_tensor("outp", [NT, 8, 128, 4, 512], bf16, kind="ExternalOutput").ap()

    with tc.tile_pool(name="persist", bufs=1) as persist, \
         tc.tile_pool(name="hidp", bufs=3) as hidp, \
         tc.tile_pool(name="qtp", bufs=1) as qtp, \
         tc.tile_pool(name="sqp", bufs=2) as sqp, \
         tc.tile_pool(name="rqp", bufs=2) as rqp, \
         tc.tile_pool(name="exp_", bufs=8) as exp_, \
         tc.tile_pool(name="pairp", bufs=3) as pairp, \
         tc.tile_pool(name="dsump", bufs=1) as dsump, \
         tc.tile_pool(name="rinvp", bufs=1) as rinvp, \
         tc.tile_pool(name="aop", bufs=2) as aop, \
         tc.tile_pool(name="obp", bufs=2) as obp, \
         tc.tile_pool(name="psA", bufs=4, space="PSUM") as psA, \
         tc.tile_pool(name="psB", bufs=2, space="PSUM") as psB, \
         tc.tile_pool(name="psS", bufs=2, space="PSUM") as psS:

        # ---- persistent weights / constants ----
        wq_sb = persist.tile([128, 4, 32, 128], bf16)
        for m in range(4):
            nc.gpsimd.dma_start(out=wq_sb[:, m], in_=wq[m])
        ones = persist.tile([128, 128], f32r)
        nc.gpsimd.dma_start(out=ones, in_=onesd)
        eps_col = persist.tile([128, 1], f32)
        nc.vector.memset(eps_col, EPS)
        k_sb = persist.tile([128, 2048], f32r)
        nc.gpsimd.dma_start(out=k_sb, in_=kpp)
        v_sb = persist.tile([128, 16, 128], bf16)
        nc.gpsimd.dma_start(out=v_sb, in_=vt)
        wo_sb = persist.tile([128, 4, 32, 128], bf16)
        for hq in range(4):
            nc.gpsimd.dma_start(out=wo_sb[:, hq], in_=wo[hq])

        # hid chunk half-tiles [16 kt each]; 3 slots: current tcn's two
        # halves + one prefetching for tcn+1
        def load_hid_half(tcn, half):
            hh = hidp.tile([128, 16, 512], bf16, tag="hid")
            for g in (2 * half, 2 * half + 1):
                nc.sync.dma_start(out=hh[:, (g % 2) * 8:(g % 2 + 1) * 8, :],
                                  in_=hid[tcn, g])
            return hh

        hts = {(0, 0): load_hid_half(0, 0), (0, 1): load_hid_half(0, 1)}

        aoT_prev = None
        obg_cur = [None]

        def oproj_mms(tcn_prev, aoT_t, mo, n):
            """n o-projection output tiles (PE) + drains, starting at mo."""
            for moi in range(mo, mo + n):
                pp = psB.tile([128, 512], f32, tag="pp2")
                for hq in range(4):
                    nc.tensor.matmul(pp, wo_sb[:, hq, moi, :], aoT_t[:, hq, :],
                                     start=(hq == 0), stop=(hq == 3))
                mg, mj = moi // 4, moi % 4
                if mj == 0:
                    obg_cur[0] = obp.tile([128, 4, 512], bf16, tag="obg",
                                          name="obg")
                nc.vector.tensor_copy(obg_cur[0][:, mj, :], pp)
                if mj == 3:
                    nc.sync.dma_start(out=outp[tcn_prev, mg], in_=obg_cur[0])

        for tcn in range(NT):
            # ---- q-projection + per-head RMSNorm (heads m = 0..3) ----
            qT = qtp.tile([128, HL, 512], f32r, tag="qT")
            for m in range(4):
                pq = psA.tile([128, 512], f32, tag="acc")
                for kt in range(32):
                    nc.tensor.matmul(pq, wq_sb[:, m, kt, :],
                                     hts[(tcn, kt // 16)][:, kt % 16, :],
                                     start=(kt == 0), stop=(kt == 31))
                # rsqrt(mean_d(q^2)+eps): drain q to SBUF (the verifier
                # rejects a DVE op reading the same PSUM AP twice), square
                # on GpSimd, partition-sum via ones-matmul (PE), then
                # exp(-0.5*ln(x)) on ACT; ln/exp share the attention exp's
                # table set so the activation table loads only once
                qraw = sqp.tile([128, 512], f32r, tag="qraw")
                nc.vector.tensor_copy(qraw, pq)
                sq = sqp.tile([128, 512], f32r, tag="sq")
                nc.gpsimd.tensor_mul(sq, qraw, qraw)
                ps = psB.tile([128, 512], f32, tag="pp2")
                nc.tensor.matmul(ps, ones, sq, start=True, stop=True)
                lns = rqp.tile([128, 512], f32, tag="rq")
                nc.scalar.activation(lns, ps, Ln, scale=1.0 / D, bias=eps_col[:])
                rinvq = rqp.tile([128, 512], f32, tag="rq")
                nc.scalar.activation(rinvq, lns, Exp, scale=-0.5)
                nc.vector.tensor_mul(qT[:, m, :], qraw, rinvq)

            # prefetch hid for tcn+1 into the slots being freed
            if tcn + 1 < NT:
                hts[(tcn + 1, 0)] = load_hid_half(tcn + 1, 0)
                hts[(tcn + 1, 1)] = load_hid_half(tcn + 1, 1)

            # ---- attention (+ interleaved oproj of previous tcn) ----
            po = [psA.tile([128, 512], f32, tag="acc", name=f"po{h}")
                  for h in range(4)]
            # dsum[:, h*512:(h+1)*512] accumulates sum_kk exp for head h
            dsum = dsump.tile([128, 4 * 512], f32r, tag="dsum")
            dsum_started = [False] * 4
            pair_pend = [None] * 4
            for kk in range(16):
                ks = k_sb[:, kk * 128:(kk + 1) * 128]
                # PE issue order per kk: s0 s1 [op x4] s2 s3 [op x4] AV0-3.
                # exp(h) WAR-serializes the scr slot pair; the oproj matmuls
                # fill what would otherwise be PE idle behind ACT.
                exs = []
                for h in range(2):
                    scr = psS.tile([128, 512], f32, tag="scr")
                    nc.tensor.matmul(scr, ks, qT[:, h, :], start=True, stop=True)
                    ex = exp_.tile([128, 512], bf16, tag="ex")
                    nc.scalar.activation(ex, scr, Exp)
                    exs.append(ex)
                if aoT_prev is not None:
                    oproj_mms(tcn - 1, aoT_prev, 2 * kk, 1)
                for h in range(2, 4):
                    scr = psS.tile([128, 512], f32, tag="scr")
                    nc.tensor.matmul(scr, ks, qT[:, h, :], start=True, stop=True)
                    ex = exp_.tile([128, 512], bf16, tag="ex")
                    nc.scalar.activation(ex, scr, Exp)
                    exs.append(ex)
                if aoT_prev is not None:
                    oproj_mms(tcn - 1, aoT_prev, 2 * kk + 1, 1)
                for h in range(4):
                    nc.tensor.matmul(po[h], v_sb[:, kk, :], exs[h],
                                     start=(kk == 0), stop=(kk == 15))
                # softmax denominator: bf16 pair tree (GpSimd, otherwise
                # idle) + f32 accumulate (DVE); stays off the PE
                for h in range(4):
                    if pair_pend[h] is None:
                        pair_pend[h] = exs[h]
                    else:
                        pr = pairp.tile([128, 512], bf16, tag="pair")
                        nc.gpsimd.tensor_add(pr, pair_pend[h], exs[h])
                        pair_pend[h] = None
                        dslice = dsum[:, h * 512:(h + 1) * 512]
                        if not dsum_started[h]:
                            nc.vector.tensor_copy(dslice, pr)
                            dsum_started[h] = True
                        else:
                            nc.vector.tensor_add(dslice, dslice, pr)

            # ---- softmax normalization + attn-out (transposed) ----
            rinv = rinvp.tile([128, 4 * 512], f32, tag="rinv")
            aoT = aop.tile([128, HL, 512], bf16, tag="aoT")
            for h in range(4):
                den = psB.tile([128, 512], f32, tag="pp2")
                nc.tensor.matmul(den, ones, dsum[:, h * 512:(h + 1) * 512],
                                 start=True, stop=True)
                nc.vector.reciprocal_approx_fast(
                    out=rinv[:, h * 512:(h + 1) * 512], in_=den)
                nc.vector.tensor_mul(aoT[:, h, :], po[h],
                                     rinv[:, h * 512:(h + 1) * 512])
            aoT_prev = aoT

        # trailing o-projection for the last t-chunk
        oproj_mms(NT - 1, aoT_prev, 0, 32)


_NC_CACHE = None


def _build():
    global _NC_CACHE
    if _NC_CACHE is None:
        nc = bacc.Bacc("TRN2", target_bir_lowering=False, debug=False,
                       num_devices=N_CORES)
        with tile.TileContext(nc) as tc:
            _kernel_body(tc)
        nc.compile()
        _NC_CACHE = nc
    return _NC_CACHE


def _prepare_in_maps(hidden_states, k, v, Wq, Wo, q_norm_w, k_norm_w):
    hs = np.asarray(hidden_states, np.float32)
    k_ = np.asarray(k, np.float32)[0]      # [K, KVH, D]
    v_ = np.asarray(v, np.float32)[0]
    Wq_ = np.asarray(Wq, np.float32)
    Wo_ = np.asarray(Wo, np.float32)
    wqn = np.asarray(q_norm_w, np.float64)
    wkn = np.asarray(k_norm_w, np.float64)

    # Fold k-RMSNorm, both norm weights, and the attention scale into k''.
    kd = k_.astype(np.float64)
    rk = 1.0 / np.sqrt((kd ** 2).mean(-1, keepdims=True) + EPS)
    kpp_full = (kd * rk * (wqn * wkn) * (D ** -0.5)).astype(np.float32)

    hidT = np.ascontiguousarray(hs.T)                                  # [E, T]
    # hid [tcn, g, p, j, t] with contraction tile kt = g*8 + j
    hid_tiles = np.ascontiguousarray(
        hidT.reshape(32, 128, 4, 512)        # [kt, p, tcn, t]
        .transpose(2, 0, 1, 3)               # [tcn, kt, p, t]
        .reshape(4, 4, 8, 128, 512)          # [tcn, g, j, p, t]
        .transpose(0, 1, 3, 2, 4)            # [tcn, g, p, j, t]
        .astype(BF))
    ones_arr = np.ones((128, 128), np.float32)

    in_maps = []
    for c in range(N_CORES):
        wqT = np.ascontiguousarray(Wq_[c * EL:(c + 1) * EL, :].T)      # [E, EL]
        wq_tiles = np.ascontiguousarray(
            wqT.reshape(32, 128, 4, 128).transpose(2, 1, 0, 3)         # [m,p,kt,c]
            .astype(BF))
        woT = np.ascontiguousarray(Wo_[:, c * EL:(c + 1) * EL].T)      # [EL, E]
        wo_tiles = np.ascontiguousarray(
            woT.reshape(4, 128, 32, 128).astype(BF))                   # [hq,p,mo,c]
        kppT = np.ascontiguousarray(kpp_full[:, c, :].T)               # [D, K]
        v_tiles = np.ascontiguousarray(
            v_[:, c, :].reshape(16, 128, 128).transpose(1, 0, 2)       # [p,kk,d]
            .astype(BF))
        in_maps.append({
            "hid": hid_tiles, "wq": wq_tiles, "kpp": kppT,
            "vt": v_tiles, "wo": wo_tiles, "ones": ones_arr,
        })
    return in_maps


def _gather(results):
    total = np.zeros((NT, 8, 128, 4, 512), np.float32)
    for r in results:
        total += np.asarray(r["outp"], dtype=np.float32)
    # outp[tcn, mg, p, mj, t] -> outT[(mg*4+mj)*128+p, tcn*512+t]
    outT = total.transpose(1, 3, 2, 0, 4).reshape(E, T)
    return np.ascontiguousarray(outT.T)


def kernel(hidden_states, k, v, Wq, Wo, q_norm_w, k_norm_w):
    nc = _build()
    in_maps = _prepare_in_maps(hidden_states, k, v, Wq, Wo, q_norm_w, k_norm_w)
    res = bass_utils.run_bass_kernel_spmd(nc, in_maps,
                                          core_ids=list(range(N_CORES)))
    return _gather(res.results)


# revision 12
# speedup vs baseline: 1.0723x; 1.0723x over previous
"""TRN2 Bass kernel for nn_CombCrossAttention (GQA cross-attention block).

Computation (T=2048, K=2048, E=4096, H=32 q-heads, KVH=8 kv-heads, D=128):
    q  = hidden @ Wq.T;  per-head RMSNorm(q) * q_norm_w
    kn = RMSNorm(k) * k_norm_w  (GQA: each kv head serves 4 q heads)
    attn = softmax(qn @ kn.T / sqrt(D)) @ v
    out  = attn @ Wo.T

Sharding: tensor-parallel over heads on 8 NeuronCores. Core c owns q-heads
4c..4c+3 (Wq rows 512c..512c+512) and kv-head c, plus Wo columns
512c..512c+512; each core emits a [T, E] partial of the o-projection and
the host sums the 8 partials (the "all-reduce").

Fully-fused single pipeline over t-chunks of 512 (tcn = 0..3); everything
is computed transposed ([feature, t]) so no on-chip transposes are needed.
Per-tcn emission order (PE executes in order; other engines trail):

    qproj m0 | norm(tcn-1): densum+recip+aoTmul | qproj m1 m2 m3 |
    attention kk=0..15 with oproj(tcn-1) matmuls interleaved

The PE runs 1568 x 512-row matmuls (~373 us at the observed ~2.15 GHz,
1 cycle/row) and everything else is scheduled to hide under it:
  - ACT: one pre-placed ACT_TABLE_LOAD of natural_log_exp_and_others
    serves ALL activations (the per-function defaults would otherwise
    reload tables 32x, 41 us); scores exp -> bf16; RMS rsqrt as
    exp(-0.5*ln(x)).
  - The softmax denominator stays off the PE: bf16 pair sums of exp
    tiles (GpSimd, otherwise idle) written into a [128,2048] tile, ONE
    f32 DVE accumulate per kk-pair, one [128,512] ones-matmul per
    (head, tcn) for the cross-partition sum, DVE fast-approx reciprocal
    (the exact DVE reciprocal is 5x slower; ACT reciprocal would force
    table switches).
  - qproj drains q to SBUF (the verifier rejects a DVE op reading the
    same PSUM AP twice), squares on GpSimd.

PSUM (8 banks) is time-shared via pool tags:
  acc  4x[128,512]  attention AV accumulators <-> qproj psum (m2, m3)
  scr  2x[128,512]  score tiles <-> qproj psum (m0, m1); exp WAR-
       serializes slot reuse, and the interleaved oproj matmuls fill
       what would otherwise be PE idle behind ACT
  pp2  2x[128,512]  oproj tiles <-> RMS mean-square sums <-> denominators

bf16 where the error budget (2e-2) allows: hidden/Wq (q-proj), v / exp
tiles (AV matmul), Wo / attn-out (o-proj), output partials; scores and
k'' stay f32r. All matmuls run at 1 cycle/row.
"""
import sys

sys.path.insert(0, "/opt/trn_rl_repo")

import numpy as np
import ml_dtypes

import jax
try:
    jax.config.update("jax_compilation_cache_dir", "/tmp/jax_neff_cache")
    jax.config.update("jax_persistent_cache_min_compile_time_secs", 1.0)
except Exception:
    pass

import concourse.bass as bass  # noqa: F401
import concourse.mybir as mybir
import concourse.tile as tile
from concourse import bacc, bass_utils

EPS = 1e-5
T, K, E, H, KVH, D = 2048, 2048, 4096, 32, 8, 128
N_CORES = 8
HL = H // N_CORES      # 4 q-heads per core
EL = HL * D            # 512 local embed columns
NT = 4                 # t-chunks of 512
f32 = mybir.dt.float32
f32r = mybir.dt.float32r
bf16 = mybir.dt.bfloat16
BF = ml_dtypes.bfloat16

Ln = mybir.ActivationFunctionType.Ln
Exp = mybir.ActivationFunctionType.Exp
# act_info.json set 6 = natural_log_exp_and_others: covers Ln + Exp + Copy
LN_EXP_SET = 6


def _kernel_body(tc):
    nc = tc.nc
    # hid: [tcn][qtr][partition][kt-in-qtr][t] bf16 (kt = qtr*4 + j)
    hid = nc.dram_tensor("hid", [NT, 8, 128, 4, 512], bf16, kind="ExternalInput").ap()
    # wq: [m][partition(e-chunk)][kt][col] bf16 lhsT tiles
    wq = nc.dram_tensor("wq", [4, 128, 32, 128], bf16, kind="ExternalInput").ap()
    kpp = nc.dram_tensor("kpp", [128, 2048], f32r, kind="ExternalInput").ap()
    vt = nc.dram_tensor("vt", [128, 16, 128], bf16, kind="ExternalInput").ap()
    # wo: [hq][partition(d)][mo][col] bf16 lhsT tiles
    wo = nc.dram_tensor("wo", [4, 128, 32, 128], bf16, kind="ExternalInput").ap()
    onesd = nc.dram_tensor("ones", [128, 128], f32r, kind="ExternalInput").ap()
    # outp: [tcn][mg][partition][mj][t] bf16 (e_global = (mg*4+mj)*128 + p)
    outp = nc.dram_tensor("outp", [NT, 8, 128, 4, 512], bf16, kind="ExternalOutput").ap()

    with tc.tile_pool(name="persist", bufs=1) as persist, \
         tc.tile_pool(name="hidp", bufs=3) as hidp, \
         tc.tile_pool(name="qtp", bufs=1) as qtp, \
         tc.tile_pool(name="sqp", bufs=2) as sqp, \
         tc.tile_pool(name="rqp", bufs=2) as rqp, \
         tc.tile_pool(name="exp_", bufs=8) as exp_, \
         tc.tile_pool(name="pairp", bufs=2) as pairp, \
         tc.tile_pool(name="dsump", bufs=1) as dsump, \
         tc.tile_pool(name="rinvp", bufs=1) as rinvp, \
         tc.tile_pool(name="aop", bufs=2) as aop, \
         tc.tile_pool(name="obp", bufs=2) as obp, \
         tc.tile_pool(name="psA", bufs=4, space="PSUM") as psA, \
         tc.tile_pool(name="psB", bufs=2, space="PSUM") as psB, \
         tc.tile_pool(name="psS", bufs=2, space="PSUM") as psS:

        # One activation-table load covering every ACT op in the kernel.
        nc.scalar.add_instruction(
            mybir.InstLoadActFuncSet(
                name=nc.get_next_instruction_name(),
                act_func_set_id=LN_EXP_SET, ins=[], outs=[]))

        # ---- persistent weights / constants ----
        # wq m0 in fine chunks first: the very first matmul needs only
        # wq[0][:, 0:8] + the first hid quarter
        wq_sb = persist.tile([128, 4, 32, 128], bf16)
        for g in range(4):
            nc.gpsimd.dma_start(out=wq_sb[:, 0, g * 8:(g + 1) * 8, :],
                                in_=wq[0, :, g * 8:(g + 1) * 8, :])
        for m in range(1, 4):
            nc.gpsimd.dma_start(out=wq_sb[:, m], in_=wq[m])
        ones = persist.tile([128, 128], f32r)
        nc.gpsimd.dma_start(out=ones, in_=onesd)
        eps_col = persist.tile([128, 1], f32)
        nc.vector.memset(eps_col, EPS)
        k_sb = persist.tile([128, 2048], f32r)
        nc.gpsimd.dma_start(out=k_sb, in_=kpp)
        v_sb = persist.tile([128, 16, 128], bf16)
        nc.gpsimd.dma_start(out=v_sb, in_=vt)
        wo_sb = persist.tile([128, 4, 32, 128], bf16)
        for hq in range(4):
            nc.gpsimd.dma_start(out=wo_sb[:, hq], in_=wo[hq])

        # hid chunk half-tiles [16 kt each]; 3 slots: current tcn's two
        # halves + one prefetching for tcn+1
        def load_hid_half(tcn, half):
            hh = hidp.tile([128, 16, 512], bf16, tag="hid", name="hh")
            for q in range(4):
                nc.sync.dma_start(out=hh[:, q * 4:(q + 1) * 4, :],
                                  in_=hid[tcn, half * 4 + q])
            return hh

        hts = {(0, 0): load_hid_half(0, 0), (0, 1): load_hid_half(0, 1)}

        state = {"aoT": None, "po": None, "dsum": None, "obg": None}

        def qproj_m(tcn, m):
            """One head's q-projection + RMSNorm chain."""
            pool, tag = (psS, "scr") if m < 2 else (psA, "acc")
            pq = pool.tile([128, 512], f32, tag=tag, name=f"pq{m}")
            for kt in range(32):
                nc.tensor.matmul(pq, wq_sb[:, m, kt, :],
                                 hts[(tcn, kt // 16)][:, kt % 16, :],
                                 start=(kt == 0), stop=(kt == 31))
            qraw = sqp.tile([128, 512], f32r, tag="qraw")
            nc.vector.tensor_copy(qraw, pq)
            sq = sqp.tile([128, 512], f32r, tag="sq")
            nc.gpsimd.tensor_mul(sq, qraw, qraw)
            ps = psB.tile([128, 512], f32, tag="pp2")
            nc.tensor.matmul(ps, ones, sq, start=True, stop=True)
            lns = rqp.tile([128, 512], f32, tag="rq")
            nc.scalar.activation(lns, ps, Ln, scale=1.0 / D, bias=eps_col[:])
            rinvq = rqp.tile([128, 512], f32, tag="rq")
            nc.scalar.activation(rinvq, lns, Exp, scale=-0.5)
            nc.vector.tensor_mul(state["qT"][:, m, :], qraw, rinvq)

        def norm_block(tcn_p):
            """Softmax denominators + 1/x + attn-out scale for chunk tcn_p."""
            dsum_t, po_t = state["dsum"], state["po"]
            rinv = rinvp.tile([128, 4 * 512], f32, tag="rinv", name="rinv")
            aoT = aop.tile([128, HL, 512], bf16, tag="aoT", name="aoT")
            for h in range(4):
                den = psB.tile([128, 512], f32, tag="pp2", name="den")
                nc.tensor.matmul(den, ones, dsum_t[:, h * 512:(h + 1) * 512],
                                 start=True, stop=True)
                nc.vector.reciprocal_approx_fast(
                    out=rinv[:, h * 512:(h + 1) * 512], in_=den)
                nc.vector.tensor_mul(aoT[:, h, :], po_t[h],
                                     rinv[:, h * 512:(h + 1) * 512])
            state["aoT"] = aoT

        def oproj_mms(tcn_prev, mo, n):
            """n o-projection output tiles (PE) + drains, starting at mo."""
            for moi in range(mo, mo + n):
                pp = psB.tile([128, 512], f32, tag="pp2", name="pp")
                for hq in range(4):
                    nc.tensor.matmul(pp, wo_sb[:, hq, moi, :],
                                     state["aoT"][:, hq, :],
                                     start=(hq == 0), stop=(hq == 3))
                mg, mj = moi // 4, moi % 4
                if mj == 0:
                    state["obg"] = obp.tile([128, 4, 512], bf16, tag="obg",
                                            name="obg")
                nc.vector.tensor_copy(state["obg"][:, mj, :], pp)
                if mj == 3:
                    nc.sync.dma_start(out=outp[tcn_prev, mg], in_=state["obg"])

        for tcn in range(NT):
            # ---- q-projection; previous chunk's normalization is wedged
            # after m0 so its DVE work hides under m0-m3's matmuls and the
            # densum matmuls never stall the PE ----
            qT = qtp.tile([128, HL, 512], f32r, tag="qT")
            state["qT"] = qT
            qproj_m(tcn, 0)
            if tcn > 0:
                norm_block(tcn - 1)
            for m in range(1, 4):
                qproj_m(tcn, m)

            # prefetch hid for tcn+1 into the slots being freed
            if tcn + 1 < NT:
                hts[(tcn + 1, 0)] = load_hid_half(tcn + 1, 0)
                hts[(tcn + 1, 1)] = load_hid_half(tcn + 1, 1)

            # ---- attention (+ interleaved oproj of previous tcn) ----
            po = [psA.tile([128, 512], f32, tag="acc", name=f"po{h}")
                  for h in range(4)]
            # dsum[:, h*512:(h+1)*512] accumulates sum_kk exp for head h
            dsum = dsump.tile([128, 4 * 512], f32r, tag="dsum")
            state["po"], state["dsum"] = po, dsum
            dsum_started = False
            pair_pend = [None] * 4
            pair4 = None
            interleave = tcn > 0
            for kk in range(16):
                ks = k_sb[:, kk * 128:(kk + 1) * 128]
                # PE order per kk: s0 s1 [op] s2 s3 [op] AV0-3 (kk=0 runs
                # the oproj matmuls first: the RMS chain of m3 needs ~5 us
                # past the end of qproj before s3 can issue)
                if interleave and kk == 0:
                    oproj_mms(tcn - 1, 0, 2)
                exs = []
                for h in range(4):
                    scr = psS.tile([128, 512], f32, tag="scr", name="scr")
                    nc.tensor.matmul(scr, ks, qT[:, h, :], start=True, stop=True)
                    ex = exp_.tile([128, 512], bf16, tag="ex", name="ex")
                    nc.scalar.activation(ex, scr, Exp)
                    exs.append(ex)
                    if interleave and h == 1 and kk > 0:
                        oproj_mms(tcn - 1, 2 * kk, 1)
                if interleave and kk > 0:
                    oproj_mms(tcn - 1, 2 * kk + 1, 1)
                for h in range(4):
                    nc.tensor.matmul(po[h], v_sb[:, kk, :], exs[h],
                                     start=(kk == 0), stop=(kk == 15))
                # softmax denominator: bf16 pair sums (GpSimd, otherwise
                # idle) into one [128,2048] tile, ONE f32 DVE accumulate
                # per kk-pair; stays off the PE
                if kk % 2 == 0:
                    pair_pend = exs
                else:
                    pair4 = pairp.tile([128, 4 * 512], bf16, tag="pair",
                                       name="pair4")
                    for h in range(4):
                        nc.gpsimd.tensor_add(pair4[:, h * 512:(h + 1) * 512],
                                             pair_pend[h], exs[h])
                    if not dsum_started:
                        nc.vector.tensor_copy(dsum, pair4)
                        dsum_started = True
                    else:
                        nc.vector.tensor_add(dsum, dsum, pair4)

        # trailing normalization + o-projection for the last t-chunk
        norm_block(NT - 1)
        oproj_mms(NT - 1, 0, 32)


_NC_CACHE = None


def _build():
    global _NC_CACHE
    if _NC_CACHE is None:
        nc = bacc.Bacc("TRN2", target_bir_lowering=False, debug=False,
                       num_devices=N_CORES)
        with tile.TileContext(nc) as tc:
            _kernel_body(tc)
        nc.compile()
        _NC_CACHE = nc
    return _NC_CACHE


def _prepare_in_maps(hidden_states, k, v, Wq, Wo, q_norm_w, k_norm_w):
    hs = np.asarray(hidden_states, np.float32)
    k_ = np.asarray(k, np.float32)[0]      # [K, KVH, D]
    v_ = np.asarray(v, np.float32)[0]
    Wq_ = np.asarray(Wq, np.float32)
    Wo_ = np.asarray(Wo, np.float32)
    wqn = np.asarray(q_norm_w, np.float64)
    wkn = np.asarray(k_norm_w, np.float64)

    # Fold k-RMSNorm, both norm weights, and the attention scale into k''.
    kd = k_.astype(np.float64)
    rk = 1.0 / np.sqrt((kd ** 2).mean(-1, keepdims=True) + EPS)
    kpp_full = (kd * rk * (wqn * wkn) * (D ** -0.5)).astype(np.float32)

    hidT = np.ascontiguousarray(hs.T)                                  # [E, T]
    # hid [tcn, qtr, p, j, t] with contraction tile kt = qtr*4 + j
    hid_tiles = np.ascontiguousarray(
        hidT.reshape(32, 128, 4, 512)        # [kt, p, tcn, t]
        .transpose(2, 0, 1, 3)               # [tcn, kt, p, t]
        .reshape(4, 8, 4, 128, 512)          # [tcn, qtr, j, p, t]
        .transpose(0, 1, 3, 2, 4)            # [tcn, qtr, p, j, t]
        .astype(BF))
    ones_arr = np.ones((128, 128), np.float32)

    in_maps = []
    for c in range(N_CORES):
        wqT = np.ascontiguousarray(Wq_[c * EL:(c + 1) * EL, :].T)      # [E, EL]
        wq_tiles = np.ascontiguousarray(
            wqT.reshape(32, 128, 4, 128).transpose(2, 1, 0, 3)         # [m,p,kt,c]
            .astype(BF))
        woT = np.ascontiguousarray(Wo_[:, c * EL:(c + 1) * EL].T)      # [EL, E]
        wo_tiles = np.ascontiguousarray(
            woT.reshape(4, 128, 32, 128).astype(BF))                   # [hq,p,mo,c]
        kppT = np.ascontiguousarray(kpp_full[:, c, :].T)               # [D, K]
        v_tiles = np.ascontiguousarray(
            v_[:, c, :].reshape(16, 128, 128).transpose(1, 0, 2)       # [p,kk,d]
            .astype(BF))
        in_maps.append({
            "hid": hid_tiles, "wq": wq_tiles, "kpp": kppT,
            "vt": v_tiles, "wo": wo_tiles, "ones": ones_arr,
        })
    return in_maps


def _gather(results):
    total = np.zeros((NT, 8, 128, 4, 512), np.float32)
    for r in results:
        total += np.asarray(r["outp"], dtype=np.float32)
    # outp[tcn, mg, p, mj, t] -> outT[(mg*4+mj)*128+p, tcn*512+t]
    outT = total.transpose(1, 3, 2, 0, 4).reshape(E, T)
    return np.ascontiguousarray(outT.T)


def kernel(hidden_states, k, v, Wq, Wo, q_norm_w, k_norm_w):
    nc = _build()
    in_maps = _prepare_in_maps(hidden_states, k, v, Wq, Wo, q_norm_w, k_norm_w)
    res = bass_utils.run_bass_kernel_spmd(nc, in_maps,
                                          core_ids=list(range(N_CORES)))
    return _gather(res.results)


# revision 17
# speedup vs baseline: 1.0795x; 1.0067x over previous
"""TRN2 Bass kernel for nn_CombCrossAttention (GQA cross-attention block).

Computation (T=2048, K=2048, E=4096, H=32 q-heads, KVH=8 kv-heads, D=128):
    q  = hidden @ Wq.T;  per-head RMSNorm(q) * q_norm_w
    kn = RMSNorm(k) * k_norm_w  (GQA: each kv head serves 4 q heads)
    attn = softmax(qn @ kn.T / sqrt(D)) @ v
    out  = attn @ Wo.T

Sharding: tensor-parallel over heads on 8 NeuronCores. Core c owns q-heads
4c..4c+3 (Wq rows 512c..512c+512) and kv-head c, plus Wo columns
512c..512c+512; each core emits a [T, E] partial of the o-projection and
the host sums the 8 partials (the "all-reduce").

Fully-fused single pipeline over t-chunks of 512 (tcn = 0..3); everything
is computed transposed ([feature, t]) so no on-chip transposes are needed.
The PE runs 1568 x 512-row matmuls (~366 us at the observed ~2.15 GHz,
1 cycle/row) and everything else is scheduled to hide under it:
  - attention(tcn) interleaves oproj(tcn-1) matmuls into the PE idle the
    exp WAR-serialization would otherwise leave; attention(0), which has
    no previous oproj, instead absorbs qproj(1)'s m0/m1 matmuls (4/kk),
    balancing its PE work against the exp chain on ACT
  - norm(tcn-1) (softmax denominators + attn-out scale) is wedged inside
    qproj(tcn) so its matmuls never wait on the DVE/GpSimd chains
  - ACT: one pre-placed ACT_TABLE_LOAD of natural_log_exp_and_others
    serves ALL activations (per-function defaults would reload tables
    32x / 41 us); scores exp -> bf16; RMS rsqrt as exp(-0.5*ln(x))
  - softmax denominator stays off the PE: bf16 pair sums of exp tiles
    (GpSimd) into a [128,2048] tile, ONE f32 DVE accumulate per kk-pair,
    one [128,512] ones-matmul per (head, tcn) for the cross-partition
    sum, DVE fast-approx reciprocal (exact DVE reciprocal is 5x slower;
    ACT reciprocal would force table switches)
  - qproj drains q to SBUF (the verifier rejects a DVE op reading the
    same PSUM AP twice) and squares on GpSimd

PSUM (8 banks) is time-shared via pool tags:
  acc  4x[128,512]  attention AV accumulators <-> qproj psum (m2, m3)
  scr  2x[128,512]  score tiles <-> qproj psum (m0, m1); exp WAR-
       serializes slot reuse, which is what creates the PE idle the
       interleaved matmuls fill
  pp2  2x[128,512]  oproj tiles <-> RMS sums <-> denominators <-> the
       tcn0-absorbed qproj(1) m0/m1 accumulators

bf16 where the error budget (2e-2) allows: hidden/Wq (q-proj), v / exp
tiles (AV matmul), Wo / attn-out (o-proj), output partials; scores and
k'' stay f32r. All matmuls run at 1 cycle/row.
"""
import sys

sys.path.insert(0, "/opt/trn_rl_repo")

import numpy as np
import ml_dtypes

import jax
try:
    jax.config.update("jax_compilation_cache_dir", "/tmp/jax_neff_cache")
    jax.config.update("jax_persistent_cache_min_compile_time_secs", 1.0)
except Exception:
    pass

import concourse.bass as bass  # noqa: F401
import concourse.mybir as mybir
import concourse.tile as tile
from concourse import bacc, bass_utils

EPS = 1e-5
T, K, E, H, KVH, D = 2048, 2048, 4096, 32, 8, 128
N_CORES = 8
HL = H // N_CORES      # 4 q-heads per core
EL = HL * D            # 512 local embed columns
NT = 4                 # t-chunks of 512
f32 = mybir.dt.float32
f32r = mybir.dt.float32r
bf16 = mybir.dt.bfloat16
BF = ml_dtypes.bfloat16

Ln = mybir.ActivationFunctionType.Ln
Exp = mybir.ActivationFunctionType.Exp
# act_info.json set 6 = natural_log_exp_and_others: covers Ln + Exp + Copy
LN_EXP_SET = 6


def _kernel_body(tc):
    nc = tc.nc
    # hid: [tcn][qtr][partition][kt-in-qtr][t] bf16 (kt = qtr*4 + j)
    hid = nc.dram_tensor("hid", [NT, 8, 128, 4, 512], bf16, kind="ExternalInput").ap()
    # wq: [m][partition(e-chunk)][kt][col] bf16 lhsT tiles
    wq = nc.dram_tensor("wq", [4, 128, 32, 128], bf16, kind="ExternalInput").ap()
    kpp = nc.dram_tensor("kpp", [128, 2048], f32r, kind="ExternalInput").ap()
    vt = nc.dram_tensor("vt", [128, 16, 128], bf16, kind="ExternalInput").ap()
    # wo: [hq][partition(d)][mo][col] bf16 lhsT tiles
    wo = nc.dram_tensor("wo", [4, 128, 32, 128], bf16, kind="ExternalInput").ap()
    onesd = nc.dram_tensor("ones", [128, 128], f32r, kind="ExternalInput").ap()
    # outp: [tcn][mg][partition][mj][t] bf16 (e_global = (mg*4+mj)*128 + p)
    outp = nc.dram_tensor("outp", [NT, 8, 128, 4, 512], bf16, kind="ExternalOutput").ap()

    with tc.tile_pool(name="persist", bufs=1) as persist, \
         tc.tile_pool(name="hidp", bufs=3) as hidp, \
         tc.tile_pool(name="qtp", bufs=1) as qtp, \
         tc.tile_pool(name="sqp", bufs=2) as sqp, \
         tc.tile_pool(name="rqp", bufs=2) as rqp, \
         tc.tile_pool(name="exp_", bufs=8) as exp_, \
         tc.tile_pool(name="pairp", bufs=2) as pairp, \
         tc.tile_pool(name="dsump", bufs=1) as dsump, \
         tc.tile_pool(name="rinvp", bufs=1) as rinvp, \
         tc.tile_pool(name="aop", bufs=2) as aop, \
         tc.tile_pool(name="obp", bufs=2) as obp, \
         tc.tile_pool(name="psA", bufs=4, space="PSUM") as psA, \
         tc.tile_pool(name="psB", bufs=2, space="PSUM") as psB, \
         tc.tile_pool(name="psS", bufs=2, space="PSUM") as psS:

        # One activation-table load covering every ACT op in the kernel.
        nc.scalar.add_instruction(
            mybir.InstLoadActFuncSet(
                name=nc.get_next_instruction_name(),
                act_func_set_id=LN_EXP_SET, ins=[], outs=[]))

        # ---- persistent weights / constants ----
        # wq m0 in fine chunks first: the very first matmul needs only
        # wq[0][:, 0:8] + the first hid quarter. wo goes on the scalar
        # queue so its 4 MB doesn't contend with wq/hid at startup.
        wq_sb = persist.tile([128, 4, 32, 128], bf16)
        for g in range(4):
            nc.gpsimd.dma_start(out=wq_sb[:, 0, g * 8:(g + 1) * 8, :],
                                in_=wq[0, :, g * 8:(g + 1) * 8, :])
        for m in range(1, 4):
            nc.gpsimd.dma_start(out=wq_sb[:, m], in_=wq[m])
        ones = persist.tile([128, 128], f32r)
        nc.gpsimd.dma_start(out=ones, in_=onesd)
        eps_col = persist.tile([128, 1], f32)
        nc.vector.memset(eps_col, EPS)
        k_sb = persist.tile([128, 2048], f32r)
        nc.gpsimd.dma_start(out=k_sb, in_=kpp)
        v_sb = persist.tile([128, 16, 128], bf16)
        nc.gpsimd.dma_start(out=v_sb, in_=vt)
        wo_sb = persist.tile([128, 4, 32, 128], bf16)
        for hq in range(4):
            nc.scalar.dma_start(out=wo_sb[:, hq], in_=wo[hq])

        # hid chunk half-tiles [16 kt each]; 3 slots: current tcn's two
        # halves + one prefetching for tcn+1
        def load_hid_half(tcn, half):
            hh = hidp.tile([128, 16, 512], bf16, tag="hid", name="hh")
            for q in range(4):
                nc.sync.dma_start(out=hh[:, q * 4:(q + 1) * 4, :],
                                  in_=hid[tcn, half * 4 + q])
            return hh

        hts = {(0, 0): load_hid_half(0, 0), (0, 1): load_hid_half(0, 1)}

        state = {"aoT": None, "po": None, "dsum": None, "obg": None,
                 "qT": None, "qT_next": None}

        def rms_chain(m, pq, qT):
            """RMSNorm scale for one head: q/sqrt(mean(q^2)+eps) -> qT."""
            qraw = sqp.tile([128, 512], f32r, tag="qraw", name="qraw")
            nc.vector.tensor_copy(qraw, pq)
            sq = sqp.tile([128, 512], f32r, tag="sq", name="sq")
            nc.gpsimd.tensor_mul(sq, qraw, qraw)
            ps = psB.tile([128, 512], f32, tag="pp2", name="ps")
            nc.tensor.matmul(ps, ones, sq, start=True, stop=True)
            lns = rqp.tile([128, 512], f32, tag="rq", name="lns")
            nc.scalar.activation(lns, ps, Ln, scale=1.0 / D, bias=eps_col[:])
            rinvq = rqp.tile([128, 512], f32, tag="rq", name="rinvq")
            nc.scalar.activation(rinvq, lns, Exp, scale=-0.5)
            nc.vector.tensor_mul(qT[:, m, :], qraw, rinvq)

        def qproj_m(tcn, m, pool, tag):
            """One head's q-projection matmuls + RMSNorm chain."""
            pq = pool.tile([128, 512], f32, tag=tag, name=f"pq{m}")
            for kt in range(32):
                nc.tensor.matmul(pq, wq_sb[:, m, kt, :],
                                 hts[(tcn, kt // 16)][:, kt % 16, :],
                                 start=(kt == 0), stop=(kt == 31))
            rms_chain(m, pq, state["qT"])

        def norm_block(tcn_p):
            """Softmax denominators + 1/x + attn-out scale for chunk tcn_p."""
            dsum_t, po_t = state["dsum"], state["po"]
            rinv = rinvp.tile([128, 4 * 512], f32, tag="rinv", name="rinv")
            aoT = aop.tile([128, HL, 512], bf16, tag="aoT", name="aoT")
            for h in range(4):
                den = psB.tile([128, 512], f32, tag="pp2", name="den")
                nc.tensor.matmul(den, ones, dsum_t[:, h * 512:(h + 1) * 512],
                                 start=True, stop=True)
                nc.vector.reciprocal_approx_fast(
                    out=rinv[:, h * 512:(h + 1) * 512], in_=den)
                nc.vector.tensor_mul(aoT[:, h, :], po_t[h],
                                     rinv[:, h * 512:(h + 1) * 512])
            state["aoT"] = aoT

        def oproj_mms(tcn_prev, mo, n):
            """n o-projection output tiles (PE) + drains, starting at mo."""
            for moi in range(mo, mo + n):
                pp = psB.tile([128, 512], f32, tag="pp2", name="pp")
                for hq in range(4):
                    nc.tensor.matmul(pp, wo_sb[:, hq, moi, :],
                                     state["aoT"][:, hq, :],
                                     start=(hq == 0), stop=(hq == 3))
                mg, mj = moi // 4, moi % 4
                if mj == 0:
                    state["obg"] = obp.tile([128, 4, 512], bf16, tag="obg",
                                            name="obg")
                nc.vector.tensor_copy(state["obg"][:, mj, :], pp)
                if mj == 3:
                    nc.sync.dma_start(out=outp[tcn_prev, mg], in_=state["obg"])

        for tcn in range(NT):
            # ---- q-projection (tcn>=1: m0/m1 were absorbed into the
            # previous attention; their RMS chains run here). norm(tcn-1)
            # is wedged between q-proj blocks so its matmuls never stall ----
            qT = qtp.tile([128, HL, 512], f32r, tag="qT")
            state["qT"] = qT
            if tcn == 0:
                qproj_m(0, 0, psS, "scr")
                for m in range(1, 4):
                    qproj_m(0, m, psS if m == 1 else psA, "scr" if m == 1 else "acc")
            elif tcn == 1:
                qproj_m(1, 2, psS, "scr")
                rms_chain(0, state["pq01"][0], qT)
                rms_chain(1, state["pq01"][1], qT)
                norm_block(0)
                qproj_m(1, 3, psS, "scr")
            else:
                qproj_m(tcn, 0, psS, "scr")
                norm_block(tcn - 1)
                qproj_m(tcn, 1, psS, "scr")
                qproj_m(tcn, 2, psA, "acc")
                qproj_m(tcn, 3, psA, "acc")

            # prefetch hid for tcn+1 into the slots being freed
            if tcn + 1 < NT:
                hts[(tcn + 1, 0)] = load_hid_half(tcn + 1, 0)
                hts[(tcn + 1, 1)] = load_hid_half(tcn + 1, 1)

            # ---- attention (+ interleaved oproj(tcn-1), or for tcn0 the
            # absorbed qproj(1) m0/m1 matmuls) ----
            po = [psA.tile([128, 512], f32, tag="acc", name=f"po{h}")
                  for h in range(4)]
            # dsum[:, h*512:(h+1)*512] accumulates sum_kk exp for head h
            dsum = dsump.tile([128, 4 * 512], f32r, tag="dsum")
            state["po"], state["dsum"] = po, dsum
            dsum_started = False
            pair_pend = [None] * 4
            interleave = tcn > 0
            if tcn == 0:
                pq01 = [psB.tile([128, 512], f32, tag="pp2", name=f"pq01_{m}")
                        for m in range(2)]
                state["pq01"] = pq01
            for kk in range(16):
                ks = k_sb[:, kk * 128:(kk + 1) * 128]
                # PE order per kk: s0 s1 [fill] s2 s3 [fill] AV0-3, where
                # fill = oproj(tcn-1) or (tcn0) qproj(1) m0/m1 matmuls.
                # kk=0 runs the fill first: the RMS chain of m3 needs ~5us
                # past the end of qproj before s3 can issue.
                if interleave and kk == 0:
                    oproj_mms(tcn - 1, 0, 2)
                exs = []
                for h in range(4):
                    scr = psS.tile([128, 512], f32, tag="scr", name="scr")
                    nc.tensor.matmul(scr, ks, qT[:, h, :], start=True, stop=True)
                    ex = exp_.tile([128, 512], bf16, tag="ex", name="ex")
                    nc.scalar.activation(ex, scr, Exp)
                    exs.append(ex)
                    if h == 1:
                        if interleave and kk > 0 and not (tcn == 3 and kk >= 14):
                            oproj_mms(tcn - 1, 2 * kk, 1)
                        if tcn == 0:
                            for kt in (2 * kk, 2 * kk + 1):
                                nc.tensor.matmul(
                                    pq01[0], wq_sb[:, 0, kt, :],
                                    hts[(1, kt // 16)][:, kt % 16, :],
                                    start=(kt == 0), stop=(kt == 31))
                if interleave and kk > 0 and not (tcn == 3 and kk >= 14):
                    oproj_mms(tcn - 1, 2 * kk + 1, 1)
                if tcn == 0:
                    for kt in (2 * kk, 2 * kk + 1):
                        nc.tensor.matmul(
                            pq01[1], wq_sb[:, 1, kt, :],
                            hts[(1, kt // 16)][:, kt % 16, :],
                            start=(kt == 0), stop=(kt == 31))
                for h in range(4):
                    nc.tensor.matmul(po[h], v_sb[:, kk, :], exs[h],
                                     start=(kk == 0), stop=(kk == 15))
                # softmax denominator: bf16 pair sums (GpSimd, otherwise
                # idle) into one [128,2048] tile, ONE f32 DVE accumulate
                # per kk-pair; stays off the PE
                if kk % 2 == 0:
                    pair_pend = exs
                else:
                    pair4 = pairp.tile([128, 4 * 512], bf16, tag="pair",
                                       name="pair4")
                    for h in range(4):
                        nc.gpsimd.tensor_add(pair4[:, h * 512:(h + 1) * 512],
                                             pair_pend[h], exs[h])
                    if not dsum_started:
                        nc.vector.tensor_copy(dsum, pair4)
                        dsum_started = True
                    else:
                        nc.vector.tensor_add(dsum, dsum, pair4)

        # tail: the 4 oproj(2) tiles held back from attention(3) cover the
        # last denominator chain, then norm(3) + trailing oproj(3)
        oproj_mms(2, 28, 4)
        norm_block(NT - 1)
        oproj_mms(NT - 1, 0, 32)


_NC_CACHE = None


def _build():
    global _NC_CACHE
    if _NC_CACHE is None:
        nc = bacc.Bacc("TRN2", target_bir_lowering=False, debug=False,
                       num_devices=N_CORES)
        with tile.TileContext(nc) as tc:
            _kernel_body(tc)
        nc.compile()
        _NC_CACHE = nc
    return _NC_CACHE


def _prepare_in_maps(hidden_states, k, v, Wq, Wo, q_norm_w, k_norm_w):
    hs = np.asarray(hidden_states, np.float32)
    k_ = np.asarray(k, np.float32)[0]      # [K, KVH, D]
    v_ = np.asarray(v, np.float32)[0]
    Wq_ = np.asarray(Wq, np.float32)
    Wo_ = np.asarray(Wo, np.float32)
    wqn = np.asarray(q_norm_w, np.float64)
    wkn = np.asarray(k_norm_w, np.float64)

    # Fold k-RMSNorm, both norm weights, and the attention scale into k''.
    kd = k_.astype(np.float64)
    rk = 1.0 / np.sqrt((kd ** 2).mean(-1, keepdims=True) + EPS)
    kpp_full = (kd * rk * (wqn * wkn) * (D ** -0.5)).astype(np.float32)

    hidT = np.ascontiguousarray(hs.T)                                  # [E, T]
    # hid [tcn, qtr, p, j, t] with contraction tile kt = qtr*4 + j
    hid_tiles = np.ascontiguousarray(
        hidT.reshape(32, 128, 4, 512)        # [kt, p, tcn, t]
        .transpose(2, 0, 1, 3)               # [tcn, kt, p, t]
        .reshape(4, 8, 4, 128, 512)          # [tcn, qtr, j, p, t]
        .transpose(0, 1, 3, 2, 4)            # [tcn, qtr, p, j, t]
        .astype(BF))
    ones_arr = np.ones((128, 128), np.float32)

    in_maps = []
    for c in range(N_CORES):
        wqT = np.ascontiguousarray(Wq_[c * EL:(c + 1) * EL, :].T)      # [E, EL]
        wq_tiles = np.ascontiguousarray(
            wqT.reshape(32, 128, 4, 128).transpose(2, 1, 0, 3)         # [m,p,kt,c]
            .astype(BF))
        woT = np.ascontiguousarray(Wo_[:, c * EL:(c + 1) * EL].T)      # [EL, E]
        wo_tiles = np.ascontiguousarray(
            woT.reshape(4, 128, 32, 128).astype(BF))                   # [hq,p,mo,c]
        kppT = np.ascontiguousarray(kpp_full[:, c, :].T)               # [D, K]
        v_tiles = np.ascontiguousarray(
            v_[:, c, :].reshape(16, 128, 128).transpose(1, 0, 2)       # [p,kk,d]
            .astype(BF))
        in_maps.append({
            "hid": hid_tiles, "wq": wq_tiles, "kpp": kppT,
            "vt": v_tiles, "wo": wo_tiles, "ones": ones_arr,
        })
    return in_maps


def _gather(results):
    total = np.zeros((NT, 8, 128, 4, 512), np.float32)
    for r in results:
        total += np.asarray(r["outp"], dtype=np.float32)
    # outp[tcn, mg, p, mj, t] -> outT[(mg*4+mj)*128+p, tcn*512+t]
    outT = total.transpose(1, 3, 2, 0, 4).reshape(E, T)
    return np.ascontiguousarray(outT.T)


def kernel(hidden_states, k, v, Wq, Wo, q_norm_w, k_norm_w):
    nc = _build()
    in_maps = _prepare_in_maps(hidden_states, k, v, Wq, Wo, q_norm_w, k_norm_w)
    res = bass_utils.run_bass_kernel_spmd(nc, in_maps,
                                          core_ids=list(range(N_CORES)))
    return _gather(res.results)


# revision 24
# speedup vs baseline: 1.0914x; 1.0110x over previous
"""TRN2 Bass kernel for nn_CombCrossAttention (GQA cross-attention block).

Computation (T=2048, K=2048, E=4096, H=32 q-heads, KVH=8 kv-heads, D=128):
    q  = hidden @ Wq.T;  per-head RMSNorm(q) * q_norm_w
    kn = RMSNorm(k) * k_norm_w  (GQA: each kv head serves 4 q heads)
    attn = softmax(qn @ kn.T / sqrt(D)) @ v
    out  = attn @ Wo.T

Sharding: tensor-parallel over heads on 8 NeuronCores. Core c owns q-heads
4c..4c+3 (Wq rows 512c..512c+512) and kv-head c, plus Wo columns
512c..512c+512; each core emits a [T, E] partial of the o-projection and
the host sums the 8 partials (the "all-reduce").

Fully-fused single pipeline over t-chunks of 512 (tcn = 0..3); everything
is computed transposed ([feature, t]) so no on-chip transposes are needed.
The PE runs 1568 x 512-row matmuls (~366 us at the observed ~2.15 GHz,
1 cycle/row) and everything else is scheduled to hide under it:
  - attention(tcn) interleaves oproj(tcn-1) matmuls into the PE idle the
    exp WAR-serialization would otherwise leave; attention(0), which has
    no previous oproj, instead absorbs qproj(1)'s m0/m1 matmuls (4/kk),
    balancing its PE work against the exp chain on ACT
  - norm(tcn-1) (softmax denominators + attn-out scale) is wedged inside
    qproj(tcn) so its matmuls never wait on the DVE/GpSimd chains
  - ACT: one pre-placed ACT_TABLE_LOAD of natural_log_exp_and_others
    serves ALL activations (per-function defaults would reload tables
    32x / 41 us); scores exp -> bf16; RMS rsqrt as exp(-0.5*ln(x))
  - softmax denominator stays off the PE: bf16 pair sums of exp tiles
    (GpSimd) into a [128,2048] tile, ONE f32 DVE accumulate per kk-pair,
    one [128,512] ones-matmul per (head, tcn) for the cross-partition
    sum, DVE fast-approx reciprocal (exact DVE reciprocal is 5x slower;
    ACT reciprocal would force table switches)
  - qproj drains q to SBUF (the verifier rejects a DVE op reading the
    same PSUM AP twice) and squares on GpSimd

PSUM (8 banks) is time-shared via pool tags:
  acc  4x[128,512]  attention AV accumulators <-> qproj psum (m2, m3)
  scr  2x[128,512]  score tiles <-> qproj psum (m0, m1); exp WAR-
       serializes slot reuse, which is what creates the PE idle the
       interleaved matmuls fill
  pp2  2x[128,512]  oproj tiles <-> RMS sums <-> denominators <-> the
       tcn0-absorbed qproj(1) m0/m1 accumulators

bf16 where the error budget (2e-2) allows: hidden/Wq (q-proj), v / exp
tiles (AV matmul), Wo / attn-out (o-proj), output partials; scores and
k'' stay f32r. All matmuls run at 1 cycle/row.
"""
import sys

sys.path.insert(0, "/opt/trn_rl_repo")

import numpy as np
import ml_dtypes

import jax
try:
    jax.config.update("jax_compilation_cache_dir", "/tmp/jax_neff_cache")
    jax.config.update("jax_persistent_cache_min_compile_time_secs", 1.0)
except Exception:
    pass

import concourse.bass as bass  # noqa: F401
import concourse.mybir as mybir
import concourse.tile as tile
from concourse import bacc, bass_utils

EPS = 1e-5
T, K, E, H, KVH, D = 2048, 2048, 4096, 32, 8, 128
N_CORES = 8
HL = H // N_CORES      # 4 q-heads per core
EL = HL * D            # 512 local embed columns
NT = 4                 # t-chunks of 512
f32 = mybir.dt.float32
f32r = mybir.dt.float32r
bf16 = mybir.dt.bfloat16
BF = ml_dtypes.bfloat16

Ln = mybir.ActivationFunctionType.Ln
Exp = mybir.ActivationFunctionType.Exp
# act_info.json set 6 = natural_log_exp_and_others: covers Ln + Exp + Copy
LN_EXP_SET = 6


def _kernel_body(tc):
    nc = tc.nc
    # hid: [tcn][qtr][partition][kt-in-qtr][t] bf16 (kt = qtr*4 + j)
    hid = nc.dram_tensor("hid", [NT, 8, 128, 4, 512], bf16, kind="ExternalInput").ap()
    # wq: [m][partition(e-chunk)][kt][col] bf16 lhsT tiles
    wq = nc.dram_tensor("wq", [4, 128, 32, 128], bf16, kind="ExternalInput").ap()
    kpp = nc.dram_tensor("kpp", [128, 2048], f32r, kind="ExternalInput").ap()
    vt = nc.dram_tensor("vt", [128, 16, 128], bf16, kind="ExternalInput").ap()
    # wo: [hq][partition(d)][mo][col] bf16 lhsT tiles
    wo = nc.dram_tensor("wo", [4, 128, 32, 128], bf16, kind="ExternalInput").ap()
    onesd = nc.dram_tensor("ones", [128, 128], f32r, kind="ExternalInput").ap()
    # outp: [tcn][mg][partition][mj][t] bf16 (e_global = (mg*4+mj)*128 + p)
    outp = nc.dram_tensor("outp", [NT, 8, 128, 4, 512], bf16, kind="ExternalOutput").ap()

    with tc.tile_pool(name="persist", bufs=1) as persist, \
         tc.tile_pool(name="hidp", bufs=3) as hidp, \
         tc.tile_pool(name="qtp", bufs=1) as qtp, \
         tc.tile_pool(name="sqp", bufs=2) as sqp, \
         tc.tile_pool(name="rqp", bufs=2) as rqp, \
         tc.tile_pool(name="exp_", bufs=8) as exp_, \
         tc.tile_pool(name="pairp", bufs=2) as pairp, \
         tc.tile_pool(name="dsump", bufs=1) as dsump, \
         tc.tile_pool(name="rinvp", bufs=1) as rinvp, \
         tc.tile_pool(name="aop", bufs=2) as aop, \
         tc.tile_pool(name="obp", bufs=2) as obp, \
         tc.tile_pool(name="psA", bufs=4, space="PSUM") as psA, \
         tc.tile_pool(name="psB", bufs=2, space="PSUM") as psB, \
         tc.tile_pool(name="psS", bufs=2, space="PSUM") as psS:

        # One activation-table load covering every ACT op in the kernel.
        nc.scalar.add_instruction(
            mybir.InstLoadActFuncSet(
                name=nc.get_next_instruction_name(),
                act_func_set_id=LN_EXP_SET, ins=[], outs=[]))

        # ---- persistent weights / constants ----
        # wq m0 in fine chunks first: the very first matmul needs only
        # wq[0][:, 0:8] + the first hid quarter. wo goes on the scalar
        # queue so its 4 MB doesn't contend with wq/hid at startup.
        wq_sb = persist.tile([128, 4, 32, 128], bf16)
        for g in range(4):
            nc.gpsimd.dma_start(out=wq_sb[:, 0, g * 8:(g + 1) * 8, :],
                                in_=wq[0, :, g * 8:(g + 1) * 8, :])
        for m in range(1, 4):
            nc.gpsimd.dma_start(out=wq_sb[:, m], in_=wq[m])
        ones = persist.tile([128, 128], f32r)
        nc.gpsimd.dma_start(out=ones, in_=onesd)
        eps_col = persist.tile([128, 1], f32)
        nc.vector.memset(eps_col, EPS)
        # k/v/wo tiles are allocated here but their DMAs are deferred into
        # the tcn0 emission so the startup window carries only wq + hid(0)
        k_sb = persist.tile([128, 2048], f32r)
        v_sb = persist.tile([128, 16, 128], bf16)
        wo_sb = persist.tile([128, 4, 32, 128], bf16)

        # hid chunk half-tiles [16 kt each]; 3 slots: current tcn's two
        # halves + one prefetching for tcn+1
        def load_hid_half(tcn, half):
            hh = hidp.tile([128, 16, 512], bf16, tag="hid", name="hh")
            for q in range(4):
                nc.sync.dma_start(out=hh[:, q * 4:(q + 1) * 4, :],
                                  in_=hid[tcn, half * 4 + q])
            return hh

        hts = {(0, 0): load_hid_half(0, 0), (0, 1): load_hid_half(0, 1)}

        state = {"aoT": None, "po": None, "dsum": None, "obg": None,
                 "qT": None, "qT_next": None}

        def rms_chain(m, pq, qT):
            """RMSNorm scale for one head: q/sqrt(mean(q^2)+eps) -> qT."""
            qraw = sqp.tile([128, 512], f32r, tag="qraw", name="qraw")
            nc.vector.tensor_copy(qraw, pq)
            sq = sqp.tile([128, 512], f32r, tag="sq", name="sq")
            nc.gpsimd.tensor_mul(sq, qraw, qraw)
            ps = psB.tile([128, 512], f32, tag="pp2", name="ps")
            nc.tensor.matmul(ps, ones, sq, start=True, stop=True)
            lns = rqp.tile([128, 512], f32, tag="rq", name="lns")
            nc.scalar.activation(lns, ps, Ln, scale=1.0 / D, bias=eps_col[:])
            rinvq = rqp.tile([128, 512], f32, tag="rq", name="rinvq")
            nc.scalar.activation(rinvq, lns, Exp, scale=-0.5)
            nc.vector.tensor_mul(qT[:, m, :], qraw, rinvq)

        def qproj_m(tcn, m, pool, tag):
            """One head's q-projection matmuls + RMSNorm chain."""
            pq = pool.tile([128, 512], f32, tag=tag, name=f"pq{m}")
            for kt in range(32):
                nc.tensor.matmul(pq, wq_sb[:, m, kt, :],
                                 hts[(tcn, kt // 16)][:, kt % 16, :],
                                 start=(kt == 0), stop=(kt == 31))
            rms_chain(m, pq, state["qT"])

        def norm_block(tcn_p):
            """Softmax denominators + 1/x + attn-out scale for chunk tcn_p."""
            dsum_t, po_t = state["dsum"], state["po"]
            rinv = rinvp.tile([128, 4 * 512], f32, tag="rinv", name="rinv")
            aoT = aop.tile([128, HL, 512], bf16, tag="aoT", name="aoT")
            for h in range(4):
                den = psB.tile([128, 512], f32, tag="pp2", name="den")
                nc.tensor.matmul(den, ones, dsum_t[:, h * 512:(h + 1) * 512],
                                 start=True, stop=True)
                nc.vector.reciprocal_approx_fast(
                    out=rinv[:, h * 512:(h + 1) * 512], in_=den)
                nc.vector.tensor_mul(aoT[:, h, :], po_t[h],
                                     rinv[:, h * 512:(h + 1) * 512])
            state["aoT"] = aoT

        def oproj_mms(tcn_prev, mo, n):
            """n o-projection output tiles (PE) + drains, starting at mo."""
            for moi in range(mo, mo + n):
                pp = psB.tile([128, 512], f32, tag="pp2", name="pp")
                for hq in range(4):
                    nc.tensor.matmul(pp, wo_sb[:, hq, moi, :],
                                     state["aoT"][:, hq, :],
                                     start=(hq == 0), stop=(hq == 3))
                mg, mj = moi // 4, moi % 4
                if mj == 0:
                    state["obg"] = obp.tile([128, 4, 512], bf16, tag="obg",
                                            name="obg")
                nc.vector.tensor_copy(state["obg"][:, mj, :], pp)
                if mj == 3:
                    nc.sync.dma_start(out=outp[tcn_prev, mg], in_=state["obg"])

        for tcn in range(NT):
            # ---- q-projection (tcn>=1: m0/m1 were absorbed into the
            # previous attention; their RMS chains run here). norm(tcn-1)
            # is wedged between q-proj blocks so its matmuls never stall ----
            qT = qtp.tile([128, HL, 512], f32r, tag="qT")
            state["qT"] = qT
            if tcn == 0:
                qproj_m(0, 0, psS, "scr")
                nc.gpsimd.dma_start(out=k_sb, in_=kpp)
                nc.gpsimd.dma_start(out=v_sb, in_=vt)
                for m in range(1, 4):
                    qproj_m(0, m, psS if m == 1 else psA, "scr" if m == 1 else "acc")
            elif tcn == 1:
                qproj_m(1, 2, psS, "scr")
                rms_chain(0, state["pq01"][0], qT)
                rms_chain(1, state["pq01"][1], qT)
                norm_block(0)
                qproj_m(1, 3, psS, "scr")
            else:
                qproj_m(tcn, 0, psS, "scr")
                norm_block(tcn - 1)
                qproj_m(tcn, 1, psS, "scr")
                qproj_m(tcn, 2, psA, "acc")
                qproj_m(tcn, 3, psA, "acc")

            # prefetch hid for tcn+1 into the slots being freed
            if tcn + 1 < NT:
                hts[(tcn + 1, 0)] = load_hid_half(tcn + 1, 0)
                hts[(tcn + 1, 1)] = load_hid_half(tcn + 1, 1)
            if tcn == 0:
                for hq in range(4):
                    nc.scalar.dma_start(out=wo_sb[:, hq], in_=wo[hq])

            # ---- attention (+ interleaved oproj(tcn-1), or for tcn0 the
            # absorbed qproj(1) m0/m1 matmuls) ----
            po = [psA.tile([128, 512], f32, tag="acc", name=f"po{h}")
                  for h in range(4)]
            # dsum[:, h*512:(h+1)*512] accumulates sum_kk exp for head h
            dsum = dsump.tile([128, 4 * 512], f32r, tag="dsum")
            state["po"], state["dsum"] = po, dsum
            dsum_started = False
            pair_pend = [None] * 4
            interleave = tcn > 0
            if tcn == 0:
                pq01 = [psB.tile([128, 512], f32, tag="pp2", name=f"pq01_{m}")
                        for m in range(2)]
                state["pq01"] = pq01
            for kk in range(16):
                ks = k_sb[:, kk * 128:(kk + 1) * 128]
                # PE order per kk: s0 s1 [fill] s2 s3 [fill] AV0-3, where
                # fill = oproj(tcn-1) or (tcn0) qproj(1) m0/m1 matmuls.
                # kk=0 runs the fill first: the RMS chain of m3 needs ~5us
                # past the end of qproj before s3 can issue.
                if interleave and kk == 0:
                    oproj_mms(tcn - 1, 0, 2)
                exs = []
                for h in range(4):
                    scr = psS.tile([128, 512], f32, tag="scr", name="scr")
                    nc.tensor.matmul(scr, ks, qT[:, h, :], start=True, stop=True)
                    ex = exp_.tile([128, 512], bf16, tag="ex", name="ex")
                    nc.scalar.activation(ex, scr, Exp)
                    exs.append(ex)
                    if h == 1:
                        if interleave and kk > 0 and not (tcn == 3 and kk >= 12):
                            oproj_mms(tcn - 1, 2 * kk, 1)
                        if tcn == 0:
                            for kt in (2 * kk, 2 * kk + 1):
                                nc.tensor.matmul(
                                    pq01[0], wq_sb[:, 0, kt, :],
                                    hts[(1, kt // 16)][:, kt % 16, :],
                                    start=(kt == 0), stop=(kt == 31))
                if interleave and kk > 0 and not (tcn == 3 and kk >= 12):
                    oproj_mms(tcn - 1, 2 * kk + 1, 1)
                if tcn == 0:
                    for kt in (2 * kk, 2 * kk + 1):
                        nc.tensor.matmul(
                            pq01[1], wq_sb[:, 1, kt, :],
                            hts[(1, kt // 16)][:, kt % 16, :],
                            start=(kt == 0), stop=(kt == 31))
                for h in range(4):
                    nc.tensor.matmul(po[h], v_sb[:, kk, :], exs[h],
                                     start=(kk == 0), stop=(kk == 15))
                # softmax denominator: bf16 pair sums (GpSimd, otherwise
                # idle) into one [128,2048] tile, ONE f32 DVE accumulate
                # per kk-pair; stays off the PE
                if kk % 2 == 0:
                    pair_pend = exs
                else:
                    pair4 = pairp.tile([128, 4 * 512], bf16, tag="pair",
                                       name="pair4")
                    # the final kk-pair of the last chunk goes on DVE: the
                    # GpSimd chain is ~5us serial and would push the last
                    # denominator past the held-back oproj cover
                    eng = nc.vector if (tcn == NT - 1 and kk == 15) else nc.gpsimd
                    for h in range(4):
                        eng.tensor_add(pair4[:, h * 512:(h + 1) * 512],
                                       pair_pend[h], exs[h])
                    if not dsum_started:
                        nc.vector.tensor_copy(dsum, pair4)
                        dsum_started = True
                    else:
                        nc.vector.tensor_add(dsum, dsum, pair4)

        # tail: the 8 oproj(2) tiles held back from attention(3) cover the
        # last denominator chain, then norm(3) + trailing oproj(3)
        oproj_mms(2, 24, 8)
        norm_block(NT - 1)
        oproj_mms(NT - 1, 0, 32)


_NC_CACHE = None


def _build():
    global _NC_CACHE
    if _NC_CACHE is None:
        nc = bacc.Bacc("TRN2", target_bir_lowering=False, debug=False,
                       num_devices=N_CORES)
        with tile.TileContext(nc) as tc:
            _kernel_body(tc)
        nc.compile()
        _NC_CACHE = nc
    return _NC_CACHE


def _prepare_in_maps(hidden_states, k, v, Wq, Wo, q_norm_w, k_norm_w):
    hs = np.asarray(hidden_states, np.float32)
    k_ = np.asarray(k, np.float32)[0]      # [K, KVH, D]
    v_ = np.asarray(v, np.float32)[0]
    Wq_ = np.asarray(Wq, np.float32)
    Wo_ = np.asarray(Wo, np.float32)
    wqn = np.asarray(q_norm_w, np.float64)
    wkn = np.asarray(k_norm_w, np.float64)

    # Fold k-RMSNorm, both norm weights, and the attention scale into k''.
    kd = k_.astype(np.float64)
    rk = 1.0 / np.sqrt((kd ** 2).mean(-1, keepdims=True) + EPS)
    kpp_full = (kd * rk * (wqn * wkn) * (D ** -0.5)).astype(np.float32)

    hidT = np.ascontiguousarray(hs.T)                                  # [E, T]
    # hid [tcn, qtr, p, j, t] with contraction tile kt = qtr*4 + j
    hid_tiles = np.ascontiguousarray(
        hidT.reshape(32, 128, 4, 512)        # [kt, p, tcn, t]
        .transpose(2, 0, 1, 3)               # [tcn, kt, p, t]
        .reshape(4, 8, 4, 128, 512)          # [tcn, qtr, j, p, t]
        .transpose(0, 1, 3, 2, 4)            # [tcn, qtr, p, j, t]
        .astype(BF))
    ones_arr = np.ones((128, 128), np.float32)

    in_maps = []
    for c in range(N_CORES):
        wqT = np.ascontiguousarray(Wq_[c * EL:(c + 1) * EL, :].T)      # [E, EL]
        wq_tiles = np.ascontiguousarray(
            wqT.reshape(32, 128, 4, 128).transpose(2, 1, 0, 3)         # [m,p,kt,c]
            .astype(BF))
        woT = np.ascontiguousarray(Wo_[:, c * EL:(c + 1) * EL].T)      # [EL, E]
        wo_tiles = np.ascontiguousarray(
            woT.reshape(4, 128, 32, 128).astype(BF))                   # [hq,p,mo,c]
        kppT = np.ascontiguousarray(kpp_full[:, c, :].T)               # [D, K]
        v_tiles = np.ascontiguousarray(
            v_[:, c, :].reshape(16, 128, 128).transpose(1, 0, 2)       # [p,kk,d]
            .astype(BF))
        in_maps.append({
            "hid": hid_tiles, "wq": wq_tiles, "kpp": kppT,
            "vt": v_tiles, "wo": wo_tiles, "ones": ones_arr,
        })
    return in_maps


def _gather(results):
    total = np.zeros((NT, 8, 128, 4, 512), np.float32)
    for r in results:
        total += np.asarray(r["outp"], dtype=np.float32)
    # outp[tcn, mg, p, mj, t] -> outT[(mg*4+mj)*128+p, tcn*512+t]
    outT = total.transpose(1, 3, 2, 0, 4).reshape(E, T)
    return np.ascontiguousarray(outT.T)


def kernel(hidden_states, k, v, Wq, Wo, q_norm_w, k_norm_w):
    nc = _build()
    in_maps = _prepare_in_maps(hidden_states, k, v, Wq, Wo, q_norm_w, k_norm_w)
    res = bass_utils.run_bass_kernel_spmd(nc, in_maps,
                                          core_ids=list(range(N_CORES)))
    return _gather(res.results)
